# revision 75
# baseline (speedup 1.0000x reference)
"""Local cross-attention Trainium2 kernel.

Strategy (8 NeuronCores, SPMD):
  - Host: queries split into 32 kd leaves of exactly 128 queries via a
    portfolio of OBLIQUE median splits (3 axes + 13 random directions per
    seed, 6 seeds), keeping the tree that minimizes the kernel's true cost
    Sum_i c_(8i+1) over sorted leaf chunk counts; each leaf gathers the
    EXACT union of its queries' neighborhoods, padded to 128-multiples.
    Rank-group i becomes slot i (KW[i] = group max), ASCENDING so only the
    largest slot's epilogue is exposed at the tail. The exact
    (chunk-key, query) 0/1 mask ships as m01.
  - Device main loop, software-pipelined over all (slot, chunk), scores
    2 chunks ahead of AV:
      scores: 8 heads via K=32 unmasked matmuls straight out of QT/KT
        (tile_position rows 32a; no QM masking tiles needed);
      E = exp(s/sqrt(32)): ONE ACT op per chunk [128, 1024] (ACT does
        essentially nothing else mid-loop);
      mask multiply vs m01: DVE 3 head-blocks + Pool 5 (Pool is SBUF-only);
      AV: ones-augmented V accumulates output + softmax denominator; each
        slot's av bank is initialized by ONE rank-1 matmul writing 1.0 on
        the never-accumulated rows (33:64, 97:128) and 0.0 elsewhere, so
        a single full-tile reciprocal stays finite;
      V copies on DVE; K projections run as w=512 emissions on the rb
        bank in early loop steps (t=1,2) where the PE pstate is ramped.
  - Epilogue per slot (staged 2/3 steps after the slot's last AV):
      ONE DVE reciprocal over the whole av bank (denominator rows 32/96
      are the only ones read); av -> av_sb (ACT); PE broadcast (1 matmul
      per parity, N=512, contraction at partition 32/96); normalize = 2
      DVE tensor_tensor ops (av_sb x rb-PSUM) into sb_OP; per-slot output
      projection into the spare half of the vps bank; y copies on DVE;
      per-slot yT DMA on sync/gpsimd.
  - Prologue: feature-group pairs load as single rearranged DMAs spread
    over sync/gpsimd/scalar queues in criticality order; no zero loads
    (dead regions are engine-memset); PE warmup matmuls hold the pstate;
    Q is projected narrow (slot-0 columns) and K narrow (chunk 0) so the
    first exp fires at ~5us.
  - kernel() runs via PJRT SPMD; if that path is unavailable it falls
    back to per-core CoreSim execution (identical program and numerics).
  - Host gathers outputs back to original query order.

CoreSim cost-model time: 32.6us/core (session baseline: 35.2us).
"""
import sys
sys.path.insert(0, '/opt/trn_rl_repo')

import numpy as np
from contextlib import ExitStack

import ml_dtypes

F = 256           # feature dim
H = 8             # heads
D = 32            # head dim
R2 = 9.0
NC = 8            # cores
P = 128
QS = 128          # queries per slot
NSLOT = 4         # slots per core (512 q / core)
NQ = NSLOT * QS

bf16 = ml_dtypes.bfloat16


# ---------------------------------------------------------------- host staging
def _leaves_dirs(cc, mask, DIRS):
    """Split all queries into 32 kd leaves of 128; at each node pick the
    median split over the given projection directions minimizing the
    children's padded key-chunk total (exact neighborhood unions)."""
    leaves = [np.arange(cc.shape[0])]
    while len(leaves) < NC * NSLOT:
        nxt = []
        for l in leaves:
            proj = cc[l] @ DIRS.T
            best = None
            for d in range(DIRS.shape[0]):
                order = np.argsort(proj[:, d], kind='stable')
                half = len(l) // 2
                l0, l1 = l[order[:half]], l[order[half:]]
                w0u = int(mask[l0].any(0).sum())
                w1u = int(mask[l1].any(0).sum())
                w0, w1 = -(-w0u // P), -(-w1u // P)
                key = (w0 + w1, max(w0, w1), w0u + w1u)
                if best is None or key < best[0]:
                    best = (key, l0, l1)
            nxt.append(best[1])
            nxt.append(best[2])
        leaves = nxt
    return leaves


def _leaves(cc, mask):
    """Portfolio of oblique-split kd trees (axis + random directions over a
    few seeds); keep the one minimizing the kernel's true cost: the sum
    over slots of the group-max padded chunk count."""
    best = None
    for seed in range(6):
        rng = np.random.default_rng(seed)
        DIRS = np.vstack([np.eye(3), rng.normal(size=(13, 3))])
        DIRS /= np.linalg.norm(DIRS, axis=1, keepdims=True)
        ls = _leaves_dirs(cc, mask, DIRS)
        chs = np.array([max(1, -(-int(mask[l].any(0).sum()) // P))
                        for l in ls])
        srt = np.sort(chs)[::-1]
        key = (int(srt[::NC].sum()), int(chs.sum()))
        if best is None or key < best[0]:
            best = (key, ls)
    return best[1]


def _plan(cc, hc):
    """kd leaves + exact-union key windows + rank-grouped slot assignment."""
    mask = np.zeros((cc.shape[0], hc.shape[0]), bool)
    for q0 in range(0, cc.shape[0], 512):
        d2 = ((cc[q0:q0+512, None, :] - hc[None, :, :]) ** 2).sum(
            -1, dtype=np.float32)
        mask[q0:q0+512] = d2 <= R2
    leaves = _leaves(cc, mask)
    sels = [np.nonzero(mask[l].any(0))[0] for l in leaves]
    chunks = np.array([max(1, (len(s) + P - 1) // P) for s in sels])
    order = np.argsort(-chunks, kind='stable')
    cores = [[] for _ in range(NC)]
    KW = []
    # ascending slot sizes: the largest slot runs LAST so only its epilogue
    # is exposed at the tail (smaller slots' epilogues hide under later
    # slots' chunks)
    for i in range(NSLOT - 1, -1, -1):
        grp = order[i * NC:(i + 1) * NC]
        KW.append(int(chunks[grp[0]]))
        for c in range(NC):
            li = grp[c]
            cores[c].append((leaves[li], sels[li]))
    return cores, KW, mask


def _stage(inputs):
    cc = np.ascontiguousarray(np.asarray(inputs['current_coords'], np.float32))
    hc = np.ascontiguousarray(np.asarray(inputs['historical_coords'], np.float32))
    cf = np.asarray(inputs['current_feats'], np.float32)
    hf = np.asarray(inputs['historical_feats'], np.float32)

    for bn in ('bq', 'bk', 'bv', 'bo'):
        assert not np.any(np.asarray(inputs[bn], np.float32)), f'{bn} nonzero'

    cores, KW, mask = _plan(cc, hc)
    NKP = sum(KW) * P          # padded key-instances per core

    WqT = np.ascontiguousarray(np.asarray(inputs['Wq'], np.float32).T).astype(bf16)
    WkT = np.ascontiguousarray(np.asarray(inputs['Wk'], np.float32).T).astype(bf16)
    WvT = np.ascontiguousarray(np.asarray(inputs['Wv'], np.float32).T).astype(bf16)
    WoT = np.ascontiguousarray(np.asarray(inputs['Wo'], np.float32).T)
    # Wo rows permuted to the epilogue's (parity, block) AV layout:
    # WoP[64*(h%2)+d, h//2, e] = Wo[e, 32*h+d]; dead rows zero.
    WoP = np.zeros((P, 4, F), np.float32)
    for h in range(H):
        rho, b = h % 2, h // 2
        WoP[64*rho:64*rho+D, b, :] = WoT[32*h:32*h+D, :]
    WoP = np.ascontiguousarray(WoP.reshape(P, 4*F)).astype(bf16)

    in_maps = []
    qmaps = []          # original query indices in slot order, per core
    for c in range(NC):
        subs = cores[c]
        qsel = np.concatenate([s[0] for s in subs])
        qmaps.append(qsel)
        kfeat = np.zeros((NKP, F), np.float32)
        m01h = np.zeros((P, NKP // P, QS), bf16)
        off = 0
        for i, (qs, sel) in enumerate(subs):
            kfeat[off:off + len(sel)] = hf[sel]
            sub = mask[np.ix_(qs, sel)].T.astype(bf16)   # [nsel, 128]
            for cix in range(KW[i]):
                lo = cix * P
                hi = min(len(sel), lo + P)
                if hi > lo:
                    m01h[0:hi - lo, off // P + cix, :] = sub[lo:hi]
            off += KW[i] * P
        in_maps.append({
            'histTf': np.ascontiguousarray(kfeat.T).astype(bf16),
            'm01': np.ascontiguousarray(m01h.reshape(P, NKP)),
            'curT': np.ascontiguousarray(cf[qsel].T).astype(bf16),
            'wqT': WqT, 'wkT': WkT, 'wvT': WvT, 'woP': WoP,
        })
    return in_maps, qmaps, KW, NKP, ()


# ---------------------------------------------------------------- bass kernel
def _build(KW, NKP, vbias=(), reps=1):
    import concourse.bass as bass
    import concourse.bacc as bacc
    import concourse.tile as tile
    from concourse import mybir

    f32 = mybir.dt.float32
    b16 = mybir.dt.bfloat16
    NCH = NKP // P
    ISCALE = 1.0 / np.sqrt(D)

    nc = bacc.Bacc("TRN2", target_bir_lowering=False, debug=False,
                   enable_asserts=False, num_devices=NC)

    t_histTf = nc.dram_tensor('histTf', [F, NKP], b16, kind='ExternalInput')
    t_m01 = nc.dram_tensor('m01', [P, NKP], b16, kind='ExternalInput')
    t_curT = nc.dram_tensor('curT', [F, NQ], b16, kind='ExternalInput')
    t_wqT = nc.dram_tensor('wqT', [F, F], b16, kind='ExternalInput')
    t_wkT = nc.dram_tensor('wkT', [F, F], b16, kind='ExternalInput')
    t_wvT = nc.dram_tensor('wvT', [F, F], b16, kind='ExternalInput')
    t_woP = nc.dram_tensor('woP', [P, 4 * F], b16, kind='ExternalInput')
    t_yT = nc.dram_tensor('yT', [F, NQ], f32, kind='ExternalOutput')

    base = np.cumsum([0] + KW)          # chunk base per slot

    with tile.TileContext(nc) as tc, ExitStack() as ctx:
        sing = ctx.enter_context(tc.tile_pool(name='sing', bufs=1))
        epool = ctx.enter_context(tc.tile_pool(name="epool", bufs=3))
        opool = ctx.enter_context(tc.tile_pool(name='opool', bufs=2))
        ps_sc = ctx.enter_context(tc.tile_pool(name='ps_sc', bufs=2, space='PSUM'))
        ps_d2 = ctx.enter_context(tc.tile_pool(name='ps_d2', bufs=1, space='PSUM'))
        ps_av = ctx.enter_context(tc.tile_pool(name='ps_av', bufs=2, space='PSUM'))
        ps_rb = ctx.enter_context(tc.tile_pool(name='ps_rb', bufs=1, space='PSUM'))

        for _rep in range(reps):
            _emit_once(nc, tc, mybir, KW, NKP, base, NCH, ISCALE,
                       sing, epool, opool, ps_sc, ps_d2, ps_av, ps_rb,
                       t_histTf, t_m01, t_curT, t_wqT, t_wkT, t_wvT,
                       t_woP, t_yT, f32, b16)

    nc.compile()
    return nc


def _emit_once(nc, tc, mybir, KW, NKP, base, NCH, ISCALE,
               sing, epool, opool, ps_sc, ps_d2, ps_av, ps_rb,
               t_histTf, t_m01, t_curT, t_wqT, t_wkT, t_wvT,
               t_woP, t_yT, f32, b16):
    Exp = mybir.ActivationFunctionType.Exp
    MSPLIT = 3          # mask multiply: DVE blocks [0:MSPLIT), Pool the rest

    # ---------------- SBUF tiles (feature-group pairs merged: [P, 2, .])
    sb_hist = sing.tile([P, 2, NKP], b16)
    sb_curT = sing.tile([P, 2, NQ], b16)
    sb_m01 = sing.tile([P, NKP], b16)
    sb_wq = sing.tile([P, 2, F], b16)
    sb_wk = sing.tile([P, 2, F], b16)
    sb_wv = sing.tile([P, 2, F], b16)
    sb_woP = sing.tile([P, 4, F], b16)
    sb_QT = [sing.tile([P, NQ], b16, tag=f'QT{g}', name=f'QT{g}')
             for g in range(2)]
    sb_KT = [sing.tile([P, NKP], b16, tag=f'KT{g}', name=f'KT{g}')
             for g in range(2)]
    sb_V = sing.tile([P, NCH, H * 33], b16)
    sb_ones = sing.tile([P, D], b16)
    sb_OP = sing.tile([P, 4, NQ], b16)
    sb_zero = sing.tile([1, 512], b16)
    sb_one512 = sing.tile([1, 512], b16)
    # init row-pattern for av banks: 1.0 on rows 33..63 (keeps the single-op
    # reciprocal over av[32:97] finite), 0.0 on all accumulated rows
    sb_init = sing.tile([1, P], b16)

    # ---------------- input DMAs first (criticality order, 3 queues);
    # memsets go to engines with no DMA-issue role
    hsp = 4 * P   # first hist piece: K proj for chunks 0..3
    tw2 = lambda t: t.ap().rearrange('(j p) k -> p j k', j=2)
    nc.sync.dma_start(out=sb_wq, in_=tw2(t_wqT))
    nc.sync.dma_start(out=sb_hist[:, :, 0:hsp], in_=tw2(t_histTf)[:, :, 0:hsp])
    hhalf = (NKP // 2 // P) * P
    nc.sync.dma_start(out=sb_m01[:, :hhalf], in_=t_m01.ap()[:, :hhalf])
    if NKP > hsp:
        nc.sync.dma_start(out=sb_hist[:, 0, hsp:],
                          in_=t_histTf.ap()[0:P, hsp:])
    nc.gpsimd.dma_start(out=sb_curT, in_=tw2(t_curT))
    nc.gpsimd.dma_start(out=sb_wk, in_=tw2(t_wkT))
    nc.gpsimd.dma_start(out=sb_m01[:, hhalf:], in_=t_m01.ap()[:, hhalf:])
    nc.sync.dma_start(out=sb_woP, in_=t_woP.ap())
    nc.scalar.dma_start(out=sb_wv, in_=tw2(t_wvT))
    if NKP > hsp:
        nc.scalar.dma_start(out=sb_hist[:, 1, hsp:],
                            in_=t_histTf.ap()[P:2*P, hsp:])

    # ---------------- memsets (small on DVE; big sb_OP fills on Pool,
    # queued after Pool's DMA issues -- needed only by the first epilogue)
    nc.vector.memset(sb_ones, 1.0)
    nc.vector.memset(sb_zero, 0.0)
    nc.vector.memset(sb_one512, 1.0)
    nc.vector.memset(sb_init, 0.0)
    nc.vector.memset(sb_init[0:1, 33:64], 1.0)
    nc.vector.memset(sb_init[0:1, 97:P], 1.0)
    # tiny dummy exp so the ACT table load runs during the DMA wait
    sb_dummy = sing.tile([1, 8], b16)
    nc.scalar.activation(sb_dummy, sb_ones[0:1, 0:8], Exp)
    nc.vector.memset(sb_V.rearrange('p c (h x) -> p c h x', h=H)[:, :, :, D:D + 1],
                     1.0)
    # sb_OP dead rows (multiplied by WoP's zeroed rows, but keep them finite)
    nc.gpsimd.memset(sb_OP[D:64, :, :], 0.0)
    nc.gpsimd.memset(sb_OP[64 + D:P, :, :], 0.0)

    # ---------------- PSUM layout
    vps = ps_d2.tile([P, 512], f32, tag='vps', name='vps')
    rb = ps_rb.tile([P, 512], f32, tag='rb', name='rb')

    def proj_ps():
        return ps_sc.tile([P, 1024], f32, tag='sc', name='ps')

    # PE warmup: keep the tensor engine busy from ~0.3us so the pstate is
    # ramped when the first real projections arrive (only sb_ones dep).
    for _ in range(100):
        nc.tensor.matmul(rb[0:D, 0:D], sb_ones[0:1, 0:D], sb_ones[0:1, 0:D],
                         start=True, stop=True, skip_group_check=True,
                         tile_position=(0, 0))

    def kt_copy(g, ps, j4, w):
        nc.vector.tensor_copy(sb_KT[g][:, j4 * P:j4 * P + w], ps)

    # Q projection for slot 0's queries only (narrow, critical path) --
    # scores(0) needs just QT[:, 0:QS] and KT chunk 0
    for g in range(2):
        ps = proj_ps()
        for j in range(2):
            nc.tensor.matmul(ps[:, 0:QS], sb_wq[:, j, g * P:(g + 1) * P],
                             sb_curT[:, j, 0:QS], start=(j == 0), stop=(j == 1))
        if g == 0:
            nc.scalar.copy(sb_QT[g][:, 0:QS], ps[:, 0:QS])
        else:
            nc.vector.tensor_copy(sb_QT[g][:, 0:QS], ps[:, 0:QS])

    # K chunk 0 narrow (critical path)
    for g in range(2):
        ps = proj_ps()
        for j in range(2):
            nc.tensor.matmul(ps[:, 0:P], sb_wk[:, j, g * P:(g + 1) * P],
                             sb_hist[:, j, 0:P], start=(j == 0), stop=(j == 1))
        kt_copy(g, ps[:, 0:P], 0, P)

    def emit_Qrest():
        # Q projection for the remaining queries; copies split ACT/DVE
        for g in range(2):
            ps = proj_ps()
            for j in range(2):
                nc.tensor.matmul(ps[:, 0:NQ - QS],
                                 sb_wq[:, j, g * P:(g + 1) * P],
                                 sb_curT[:, j, QS:], start=(j == 0),
                                 stop=(j == 1))
            if g == 0:
                nc.scalar.copy(sb_QT[g][:, QS:], ps[:, 0:NQ - QS])
            else:
                nc.vector.tensor_copy(sb_QT[g][:, QS:], ps[:, 0:NQ - QS])

    # K chunks 1..3 on vps
    w0b = min(3, NCH - 1) * P
    for g in range(2):
        ps = vps[:, 0:w0b]
        for j in range(2):
            nc.tensor.matmul(ps, sb_wk[:, j, g * P:(g + 1) * P],
                             sb_hist[:, j, P:P + w0b],
                             start=(j == 0), stop=(j == 1),
                             skip_group_check=True)
        kt_copy(g, ps, 1, w0b)

    def emit_K(j4):
        # uses the rb bank; all emissions land before the first epilogue
        # broadcast writes rb (K(4) in prologue, rest at t%4==2)
        w = min(4, NCH - j4) * P
        for g in range(2):
            ps = rb[:, 0:w]
            for j in range(2):
                nc.tensor.matmul(ps, sb_wk[:, j, g * P:(g + 1) * P],
                                 sb_hist[:, j, j4 * P:j4 * P + w],
                                 start=(j == 0), stop=(j == 1),
                                 skip_group_check=True)
            kt_copy(g, ps, j4, w)

    # remaining K projections are emitted in early loop steps (t=0,1,2)
    # where the PE pstate is ramped; rb region deps serialize them, and
    # all land before the first epilogue broadcast touches rb (t>=3)

    def emit_V(c):
        ps = vps[:, 0:F]
        for g in range(2):
            nc.tensor.matmul(ps, sb_hist[:, g, c * P:(c + 1) * P],
                             sb_wv[:, g, :], start=(g == 0), stop=(g == 1),
                             skip_group_check=True)
        vv = sb_V[:, c, :].rearrange('p (h x) -> p h x', h=H)
        if c % 2 == 0:
            nc.scalar.copy(vv[:, :, 0:D],
                           ps.rearrange('p (h x) -> p h x', h=H))
        else:
            nc.vector.tensor_copy(vv[:, :, 0:D],
                                  ps.rearrange('p (h x) -> p h x', h=H))

    # ---------------- main loop: software-pipelined over all (slot, chunk)
    av_tiles = {}
    chunks = [(s, j) for s in range(len(KW)) for j in range(KW[s])]
    n = len(chunks)
    sc_tiles = {}
    e_tiles = {}
    pending_epi = []

    def emit_S(t):
        s, j = chunks[t]
        qsl = slice(s * QS, (s + 1) * QS)
        kc = (base[s] + j) * P
        ksl = slice(kc, kc + P)
        # scores: 8 heads, K=32 unmasked, head (g, a) contracts KT/QT rows
        # 32a..32a+32 of group g; banks alternate via a-order (0,2,1,3).
        sc = ps_sc.tile([P, 1024], f32, tag='sc', name='sc')
        scv = sc.rearrange('p (b g c q) -> p b g c q', b=2, g=2, c=2)
        for g in range(2):
            for a in (0, 2, 1, 3):
                b, c = a // 2, a % 2
                nc.tensor.matmul(
                    scv[:, b, g, c, :],
                    sb_KT[g][32 * a:32 * a + 32, ksl],
                    sb_QT[g][32 * a:32 * a + 32, qsl],
                    start=True, stop=True,
                    tile_position=(32 * a, 0))
        sc_tiles[t] = sc

    def emit_EM(t):
        s, j = chunks[t]
        sc = sc_tiles.pop(t)
        e = epool.tile([P, 2, 2, 2, P], b16, tag='e', name='e')
        nc.scalar.activation(e, sc, Exp, scale=ISCALE)
        ef = e.rearrange('p b g c q -> p (b g c) q')
        kc = (base[s] + j) * P
        msl = sb_m01[:, None, kc:kc + P]
        if MSPLIT > 0:
            nc.vector.tensor_tensor(ef[:, 0:MSPLIT, :], ef[:, 0:MSPLIT, :],
                                    msl.to_broadcast([P, MSPLIT, P]),
                                    mybir.AluOpType.mult)
        nc.gpsimd.tensor_tensor(ef[:, MSPLIT:8, :], ef[:, MSPLIT:8, :],
                                msl.to_broadcast([P, 8 - MSPLIT, P]),
                                mybir.AluOpType.mult)
        e_tiles[t] = e

    def emit_AV(t):
        s, j = chunks[t]
        if j == 0:
            # zero the whole av bank and set every has_written bit so the 8
            # interleaved per-head accumulation chains can run start=False;
            # rows 33:64 get 1.0 so the single-op reciprocal over av[32:97]
            # stays finite (those rows' reciprocals are never read).
            av = av_tiles[s] = ps_av.tile([P, 512], f32, tag='av', name='av')
            nc.tensor.matmul(av, sb_init[0:1, :], sb_one512,
                             start=True, stop=False, skip_group_check=True)
        av = av_tiles[s]
        e = e_tiles.pop(t)
        nkc = KW[s]
        for h in range(H):
            g, a = divmod(h, 4)
            po = 64 * (h % 2)
            fo = 128 * (h // 2)
            nc.tensor.matmul(
                av[po:po + 33, fo:fo + QS],
                sb_V[:, base[s] + j, 33 * h:33 * h + 33],
                e[:, a // 2, g, a % 2, :],
                start=False,
                stop=(j == nkc - 1 and h == H - 1),
                skip_group_check=True,
                tile_position=(0, po))
        if j == nkc - 1:
            pending_epi.append([s, 0, 0])

    avsb_tiles = {}
    rbs_tiles = {}
    SLAST = len(KW) - 1

    def emit_epi_a(s):
        # av -> SBUF (ACT), reciprocal of denominator rows (DVE, writing
        # partitions 32/96). Hidden slots broadcast the reciprocal rows via
        # an SBUF->SBUF DMA (latency hides under later chunks); the last
        # slot uses the PE broadcast (short latency for the exposed tail).
        av = av_tiles[s]
        rec = opool.tile([P, 512], b16, tag='rec', name='rec')
        av_sb = opool.tile([P, 512], b16, tag='avsb', name='av_sb')
        with nc.allow_low_precision(reason='softmax denom reciprocal in '
                                    'bf16; rel tol 2e-2 dominates'):
            # one op covers both parities' denominator rows (32 and 96);
            # all other rows are finite (ones-filled / head values) and
            # their reciprocals are never read.
            nc.vector.reciprocal(rec, av)
        nc.scalar.copy(av_sb, av)
        for rho in range(2):
            pr = 32 + 64 * rho
            nc.tensor.matmul(
                rb[64 * rho:64 * rho + D, 0:512],
                sb_ones[pr:pr + 1, 0:D],
                rec[pr:pr + 1, 0:512],
                start=True, stop=True,
                tile_position=(pr, 64 * rho))
        avsb_tiles[s] = av_sb
        av_tiles.pop(s)

    def emit_epi_b(s):
        # normalize: sb_OP = av_sb * rb (single PSUM operand), then the
        # output projection into the spare half of vps
        av_sb = avsb_tiles.pop(s)
        halves = 1
        hq = QS // halves
        for qh in range(halves):
            qsl = slice(s * QS + qh * hq, s * QS + (qh + 1) * hq)
            hs = slice(qh * hq, (qh + 1) * hq)
            for rho in range(2):
                r0 = 64 * rho
                nc.vector.tensor_tensor(
                    sb_OP[r0:r0 + D, :, qsl],
                    av_sb[r0:r0 + D, :].rearrange(
                        'p (b q) -> p b q', b=4)[:, :, hs],
                    rb[r0:r0 + D, :].rearrange(
                        'p (b q) -> p b q', b=4)[:, :, hs],
                    mybir.AluOpType.mult)
            for g2 in range(2):
                ps = vps[:, 256 + 128 * g2:384 + 128 * g2][:, hs]
                for b in range(4):
                    nc.tensor.matmul(ps, sb_woP[:, b, g2 * P:(g2 + 1) * P],
                                     sb_OP[:, b, qsl],
                                     start=(b == 0), stop=(b == 3),
                                     skip_group_check=True)
            if halves == 2:
                emit_epi_c(s, qh, halves)

    def emit_epi_c(s, qh=0, halves=1):
        hq = QS // halves
        for g2 in range(2):
            hs = slice(qh * hq, (qh + 1) * hq)
            qsl = slice(s * QS + qh * hq, s * QS + (qh + 1) * hq)
            ps = vps[:, 256 + 128 * g2:384 + 128 * g2][:, hs]
            y = opool.tile([P, hq], f32, tag=f'y{g2}{qh}', name=f'y{g2}{qh}')
            nc.vector.tensor_copy(y, ps)
            (nc.sync if g2 == 0 else nc.gpsimd).dma_start(
                out=t_yT.ap()[g2 * P:(g2 + 1) * P, qsl], in_=y)

    emit_V(0)
    emit_V(1)
    for t in range(n + 4):
        for ent in pending_epi:
            ent[1] += 1
            if ent[2] == 0 and (ent[1] >= 2 or t >= n):
                emit_epi_a(ent[0])
                ent[2] = 1
        if t < n:
            emit_S(t)
        if 1 <= t <= n:
            emit_EM(t - 1)
        if t == 1:
            emit_Qrest()
            emit_K(4)
        if t == 2:
            for j4 in range(8, NCH, 4):
                emit_K(j4)
        if t + 2 < n:
            emit_V(t + 2)
        if t >= 2 and t - 2 < n:
            emit_AV(t - 2)
        for ent in list(pending_epi):
            if ent[2] == 1 and (ent[1] >= 3 or t >= n + 1):
                emit_epi_b(ent[0])
                emit_epi_c(ent[0])
                pending_epi.remove(ent)


_CACHE = {}


def kernel(**inputs):
    from concourse import bass_utils

    in_maps, qmaps, KW, NKP, vbias = _stage(inputs)
    key = (tuple(KW), vbias)
    if key not in _CACHE:
        _CACHE[key] = _build(KW, NKP, vbias)
    nc = _CACHE[key]
    try:
        res = bass_utils.run_bass_kernel_spmd(nc, in_maps,
                                              core_ids=list(range(NC)))
        yts = [res.results[c]['yT'] for c in range(NC)]
    except Exception:
        # PJRT path unavailable: execute per core on the instruction-level
        # simulator (same program, exact numerics)
        from concourse.bass_interp import CoreSim
        yts = []
        for c in range(NC):
            sim = CoreSim(nc, trace=False, core_id=c, publish_trace=False)
            for name, val in in_maps[c].items():
                sim.tensor(name)[:] = val
            sim.simulate(check_with_hw=False)
            yts.append(np.asarray(sim.tensor('yT')).copy())
    N = inputs['current_feats'].shape[0]
    out = np.zeros((N, F), np.float32)
    for c in range(NC):
        out[qmaps[c]] = yts[c].T
    return out


if __name__ == '__main__':
    pass


# revision 76
# speedup vs baseline: 1.0021x; 1.0021x over previous
"""Local cross-attention Trainium2 kernel.

Strategy (8 NeuronCores, SPMD):
  - Host: queries split into 32 kd leaves of exactly 128 queries via a
    portfolio of OBLIQUE median splits (3 axes + 13 random directions per
    seed, 6 seeds), keeping the tree that minimizes the kernel's true cost
    Sum_i c_(8i+1) over sorted leaf chunk counts; each leaf gathers the
    EXACT union of its queries' neighborhoods, padded to 128-multiples.
    Rank-group i becomes slot i (KW[i] = group max), ASCENDING so only the
    largest slot's epilogue is exposed at the tail. The exact
    (chunk-key, query) 0/1 mask ships as m01.
  - Device main loop, software-pipelined over all (slot, chunk), scores
    2 chunks ahead of AV:
      scores: 8 heads via K=32 unmasked matmuls straight out of QT/KT
        (tile_position rows 32a; no QM masking tiles needed);
      E = exp(s/sqrt(32)): ONE ACT op per chunk [128, 1024] (ACT does
        essentially nothing else mid-loop);
      mask multiply vs m01: DVE 3 head-blocks + Pool 5 (Pool is SBUF-only);
      AV: ones-augmented V accumulates output + softmax denominator; each
        slot's av bank is initialized by ONE rank-1 matmul writing 1.0 on
        the never-accumulated rows (33:64, 97:128) and 0.0 elsewhere, so
        a single full-tile reciprocal stays finite;
      V copies on DVE; K projections run as w=512 emissions on the rb
        bank in early loop steps (t=1,2) where the PE pstate is ramped.
  - Epilogue per slot (staged 2/3 steps after the slot's last AV):
      ONE DVE reciprocal over the whole av bank (denominator rows 32/96
      are the only ones read); av -> av_sb (ACT); PE broadcast (1 matmul
      per parity, N=512, contraction at partition 32/96); normalize = 2
      DVE tensor_tensor ops (av_sb x rb-PSUM) into sb_OP; per-slot output
      projection into the spare half of the vps bank; y copies on DVE;
      per-slot yT DMA on sync/gpsimd.
  - Prologue: feature-group pairs load as single rearranged DMAs spread
    over sync/gpsimd/scalar queues in criticality order; no zero loads
    (dead regions are engine-memset); PE warmup matmuls hold the pstate;
    Q is projected narrow (slot-0 columns) and K narrow (chunk 0) so the
    first exp fires at ~5us.
  - kernel() runs via PJRT SPMD; if that path is unavailable it falls
    back to per-core CoreSim execution (identical program and numerics).
  - Host gathers outputs back to original query order.

CoreSim cost-model time: 32.6us/core (session baseline: 35.2us).
"""
import sys
sys.path.insert(0, '/opt/trn_rl_repo')

import numpy as np
from contextlib import ExitStack

import ml_dtypes

F = 256           # feature dim
H = 8             # heads
D = 32            # head dim
R2 = 9.0
NC = 8            # cores
P = 128
QS = 128          # queries per slot
NSLOT = 4         # slots per core (512 q / core)
NQ = NSLOT * QS

bf16 = ml_dtypes.bfloat16


# ---------------------------------------------------------------- host staging
def _leaves_dirs(cc, mask, DIRS):
    """Split all queries into 32 kd leaves of 128; at each node pick the
    median split over the given projection directions minimizing the
    children's padded key-chunk total (exact neighborhood unions)."""
    leaves = [np.arange(cc.shape[0])]
    while len(leaves) < NC * NSLOT:
        nxt = []
        for l in leaves:
            proj = cc[l] @ DIRS.T
            best = None
            for d in range(DIRS.shape[0]):
                order = np.argsort(proj[:, d], kind='stable')
                half = len(l) // 2
                l0, l1 = l[order[:half]], l[order[half:]]
                w0u = int(mask[l0].any(0).sum())
                w1u = int(mask[l1].any(0).sum())
                w0, w1 = -(-w0u // P), -(-w1u // P)
                key = (w0 + w1, max(w0, w1), w0u + w1u)
                if best is None or key < best[0]:
                    best = (key, l0, l1)
            nxt.append(best[1])
            nxt.append(best[2])
        leaves = nxt
    return leaves


def _leaves(cc, mask):
    """Portfolio of oblique-split kd trees (axis + random directions over a
    few seeds); keep the one minimizing the kernel's true cost: the sum
    over slots of the group-max padded chunk count."""
    best = None
    for seed in range(6):
        rng = np.random.default_rng(seed)
        DIRS = np.vstack([np.eye(3), rng.normal(size=(13, 3))])
        DIRS /= np.linalg.norm(DIRS, axis=1, keepdims=True)
        ls = _leaves_dirs(cc, mask, DIRS)
        chs = np.array([max(1, -(-int(mask[l].any(0).sum()) // P))
                        for l in ls])
        srt = np.sort(chs)[::-1]
        key = (int(srt[::NC].sum()), int(chs.sum()))
        if best is None or key < best[0]:
            best = (key, ls)
    return best[1]


def _plan(cc, hc):
    """kd leaves + exact-union key windows + rank-grouped slot assignment."""
    mask = np.zeros((cc.shape[0], hc.shape[0]), bool)
    for q0 in range(0, cc.shape[0], 512):
        d2 = ((cc[q0:q0+512, None, :] - hc[None, :, :]) ** 2).sum(
            -1, dtype=np.float32)
        mask[q0:q0+512] = d2 <= R2
    leaves = _leaves(cc, mask)
    sels = [np.nonzero(mask[l].any(0))[0] for l in leaves]
    chunks = np.array([max(1, (len(s) + P - 1) // P) for s in sels])
    order = np.argsort(-chunks, kind='stable')
    cores = [[] for _ in range(NC)]
    KW = []
    # ascending slot sizes: the largest slot runs LAST so only its epilogue
    # is exposed at the tail (smaller slots' epilogues hide under later
    # slots' chunks)
    for i in range(NSLOT - 1, -1, -1):
        grp = order[i * NC:(i + 1) * NC]
        KW.append(int(chunks[grp[0]]))
        for c in range(NC):
            li = grp[c]
            cores[c].append((leaves[li], sels[li]))
    return cores, KW, mask


def _stage(inputs):
    cc = np.ascontiguousarray(np.asarray(inputs['current_coords'], np.float32))
    hc = np.ascontiguousarray(np.asarray(inputs['historical_coords'], np.float32))
    cf = np.asarray(inputs['current_feats'], np.float32)
    hf = np.asarray(inputs['historical_feats'], np.float32)

    for bn in ('bq', 'bk', 'bv', 'bo'):
        assert not np.any(np.asarray(inputs[bn], np.float32)), f'{bn} nonzero'

    cores, KW, mask = _plan(cc, hc)
    NKP = sum(KW) * P          # padded key-instances per core

    WqT = np.ascontiguousarray(np.asarray(inputs['Wq'], np.float32).T).astype(bf16)
    WkT = np.ascontiguousarray(np.asarray(inputs['Wk'], np.float32).T).astype(bf16)
    WvT = np.ascontiguousarray(np.asarray(inputs['Wv'], np.float32).T).astype(bf16)
    WoT = np.ascontiguousarray(np.asarray(inputs['Wo'], np.float32).T)
    # Wo rows permuted to the epilogue's (parity, block) AV layout:
    # WoP[64*(h%2)+d, h//2, e] = Wo[e, 32*h+d]; dead rows zero.
    WoP = np.zeros((P, 4, F), np.float32)
    for h in range(H):
        rho, b = h % 2, h // 2
        WoP[64*rho:64*rho+D, b, :] = WoT[32*h:32*h+D, :]
    WoP = np.ascontiguousarray(WoP.reshape(P, 4*F)).astype(bf16)

    in_maps = []
    qmaps = []          # original query indices in slot order, per core
    for c in range(NC):
        subs = cores[c]
        qsel = np.concatenate([s[0] for s in subs])
        qmaps.append(qsel)
        kfeat = np.zeros((NKP, F), np.float32)
        m01h = np.zeros((P, NKP // P, QS), bf16)
        off = 0
        for i, (qs, sel) in enumerate(subs):
            kfeat[off:off + len(sel)] = hf[sel]
            sub = mask[np.ix_(qs, sel)].T.astype(bf16)   # [nsel, 128]
            for cix in range(KW[i]):
                lo = cix * P
                hi = min(len(sel), lo + P)
                if hi > lo:
                    m01h[0:hi - lo, off // P + cix, :] = sub[lo:hi]
            off += KW[i] * P
        in_maps.append({
            'histTf': np.ascontiguousarray(kfeat.T).astype(bf16),
            'm01': np.ascontiguousarray(m01h.reshape(P, NKP)),
            'curT': np.ascontiguousarray(cf[qsel].T).astype(bf16),
            'wqT': WqT, 'wkT': WkT, 'wvT': WvT, 'woP': WoP,
        })
    return in_maps, qmaps, KW, NKP, ()


# ---------------------------------------------------------------- bass kernel
def _build(KW, NKP, vbias=(), reps=1):
    import concourse.bass as bass
    import concourse.bacc as bacc
    import concourse.tile as tile
    from concourse import mybir

    f32 = mybir.dt.float32
    b16 = mybir.dt.bfloat16
    NCH = NKP // P
    ISCALE = 1.0 / np.sqrt(D)

    nc = bacc.Bacc("TRN2", target_bir_lowering=False, debug=False,
                   enable_asserts=False, num_devices=NC)

    t_histTf = nc.dram_tensor('histTf', [F, NKP], b16, kind='ExternalInput')
    t_m01 = nc.dram_tensor('m01', [P, NKP], b16, kind='ExternalInput')
    t_curT = nc.dram_tensor('curT', [F, NQ], b16, kind='ExternalInput')
    t_wqT = nc.dram_tensor('wqT', [F, F], b16, kind='ExternalInput')
    t_wkT = nc.dram_tensor('wkT', [F, F], b16, kind='ExternalInput')
    t_wvT = nc.dram_tensor('wvT', [F, F], b16, kind='ExternalInput')
    t_woP = nc.dram_tensor('woP', [P, 4 * F], b16, kind='ExternalInput')
    t_yT = nc.dram_tensor('yT', [F, NQ], f32, kind='ExternalOutput')

    base = np.cumsum([0] + KW)          # chunk base per slot

    with tile.TileContext(nc) as tc, ExitStack() as ctx:
        sing = ctx.enter_context(tc.tile_pool(name='sing', bufs=1))
        epool = ctx.enter_context(tc.tile_pool(name="epool", bufs=3))
        opool = ctx.enter_context(tc.tile_pool(name='opool', bufs=2))
        ps_sc = ctx.enter_context(tc.tile_pool(name='ps_sc', bufs=2, space='PSUM'))
        ps_d2 = ctx.enter_context(tc.tile_pool(name='ps_d2', bufs=1, space='PSUM'))
        ps_av = ctx.enter_context(tc.tile_pool(name='ps_av', bufs=2, space='PSUM'))
        ps_rb = ctx.enter_context(tc.tile_pool(name='ps_rb', bufs=1, space='PSUM'))

        for _rep in range(reps):
            _emit_once(nc, tc, mybir, KW, NKP, base, NCH, ISCALE,
                       sing, epool, opool, ps_sc, ps_d2, ps_av, ps_rb,
                       t_histTf, t_m01, t_curT, t_wqT, t_wkT, t_wvT,
                       t_woP, t_yT, f32, b16)

    nc.compile()
    return nc


def _emit_once(nc, tc, mybir, KW, NKP, base, NCH, ISCALE,
               sing, epool, opool, ps_sc, ps_d2, ps_av, ps_rb,
               t_histTf, t_m01, t_curT, t_wqT, t_wkT, t_wvT,
               t_woP, t_yT, f32, b16):
    Exp = mybir.ActivationFunctionType.Exp
    MSPLIT = 3          # mask multiply: DVE blocks [0:MSPLIT), Pool the rest

    # ---------------- SBUF tiles (feature-group pairs merged: [P, 2, .])
    sb_hist = sing.tile([P, 2, NKP], b16)
    sb_curT = sing.tile([P, 2, NQ], b16)
    sb_m01 = sing.tile([P, NKP], b16)
    sb_wq = sing.tile([P, 2, F], b16)
    sb_wk = sing.tile([P, 2, F], b16)
    sb_wv = sing.tile([P, 2, F], b16)
    sb_woP = sing.tile([P, 4, F], b16)
    sb_QT = [sing.tile([P, NQ], b16, tag=f'QT{g}', name=f'QT{g}')
             for g in range(2)]
    sb_KT = [sing.tile([P, NKP], b16, tag=f'KT{g}', name=f'KT{g}')
             for g in range(2)]
    sb_V = sing.tile([P, NCH, H * 33], b16)
    sb_ones = sing.tile([P, D], b16)
    sb_OP = sing.tile([P, 4, NQ], b16)
    sb_zero = sing.tile([1, 512], b16)
    sb_one512 = sing.tile([1, 512], b16)
    # init row-pattern for av banks: 1.0 on rows 33..63 (keeps the single-op
    # reciprocal over av[32:97] finite), 0.0 on all accumulated rows
    sb_init = sing.tile([1, P], b16)

    # ---------------- input DMAs first (criticality order, 3 queues);
    # memsets go to engines with no DMA-issue role
    hsp = 4 * P   # first hist piece: K proj for chunks 0..3
    tw2 = lambda t: t.ap().rearrange('(j p) k -> p j k', j=2)
    nc.sync.dma_start(out=sb_wq, in_=tw2(t_wqT))
    nc.sync.dma_start(out=sb_hist[:, :, 0:hsp], in_=tw2(t_histTf)[:, :, 0:hsp])
    hhalf = (NKP // 2 // P) * P
    nc.sync.dma_start(out=sb_m01[:, :hhalf], in_=t_m01.ap()[:, :hhalf])
    if NKP > hsp:
        nc.sync.dma_start(out=sb_hist[:, 0, hsp:],
                          in_=t_histTf.ap()[0:P, hsp:])
    nc.gpsimd.dma_start(out=sb_curT, in_=tw2(t_curT))
    nc.gpsimd.dma_start(out=sb_wk, in_=tw2(t_wkT))
    nc.gpsimd.dma_start(out=sb_m01[:, hhalf:], in_=t_m01.ap()[:, hhalf:])
    nc.sync.dma_start(out=sb_woP, in_=t_woP.ap())
    nc.scalar.dma_start(out=sb_wv, in_=tw2(t_wvT))
    if NKP > hsp:
        nc.scalar.dma_start(out=sb_hist[:, 1, hsp:],
                            in_=t_histTf.ap()[P:2*P, hsp:])

    # ---------------- memsets (small on DVE; big sb_OP fills on Pool,
    # queued after Pool's DMA issues -- needed only by the first epilogue)
    nc.vector.memset(sb_ones, 1.0)
    nc.vector.memset(sb_zero, 0.0)
    nc.vector.memset(sb_one512, 1.0)
    nc.vector.memset(sb_init, 0.0)
    nc.vector.memset(sb_init[0:1, 33:64], 1.0)
    nc.vector.memset(sb_init[0:1, 97:P], 1.0)
    # tiny dummy exp so the ACT table load runs during the DMA wait
    sb_dummy = sing.tile([1, 8], b16)
    nc.scalar.activation(sb_dummy, sb_ones[0:1, 0:8], Exp)
    nc.vector.memset(sb_V.rearrange('p c (h x) -> p c h x', h=H)[:, :, :, D:D + 1],
                     1.0)
    # sb_OP dead rows (multiplied by WoP's zeroed rows, but keep them finite)
    nc.gpsimd.memset(sb_OP[D:64, :, :], 0.0)
    nc.gpsimd.memset(sb_OP[64 + D:P, :, :], 0.0)

    # ---------------- PSUM layout
    vps = ps_d2.tile([P, 512], f32, tag='vps', name='vps')
    rb = ps_rb.tile([P, 512], f32, tag='rb', name='rb')

    def proj_ps():
        return ps_sc.tile([P, 1024], f32, tag='sc', name='ps')

    # PE warmup: keep the tensor engine busy from ~0.3us so the pstate is
    # ramped when the first real projections arrive (only sb_ones dep).
    for _ in range(88):
        nc.tensor.matmul(rb[0:D, 0:D], sb_ones[0:1, 0:D], sb_ones[0:1, 0:D],
                         start=True, stop=True, skip_group_check=True,
                         tile_position=(0, 0))

    def kt_copy(g, ps, j4, w):
        nc.vector.tensor_copy(sb_KT[g][:, j4 * P:j4 * P + w], ps)

    # Q projection for slot 0's queries only (narrow, critical path) --
    # scores(0) needs just QT[:, 0:QS] and KT chunk 0
    for g in range(2):
        ps = proj_ps()
        for j in range(2):
            nc.tensor.matmul(ps[:, 0:QS], sb_wq[:, j, g * P:(g + 1) * P],
                             sb_curT[:, j, 0:QS], start=(j == 0), stop=(j == 1))
        if g == 0:
            nc.scalar.copy(sb_QT[g][:, 0:QS], ps[:, 0:QS])
        else:
            nc.vector.tensor_copy(sb_QT[g][:, 0:QS], ps[:, 0:QS])

    # K chunk 0 narrow (critical path)
    for g in range(2):
        ps = proj_ps()
        for j in range(2):
            nc.tensor.matmul(ps[:, 0:P], sb_wk[:, j, g * P:(g + 1) * P],
                             sb_hist[:, j, 0:P], start=(j == 0), stop=(j == 1))
        kt_copy(g, ps[:, 0:P], 0, P)

    def emit_Qrest():
        # Q projection for the remaining queries; copies split ACT/DVE
        for g in range(2):
            ps = proj_ps()
            for j in range(2):
                nc.tensor.matmul(ps[:, 0:NQ - QS],
                                 sb_wq[:, j, g * P:(g + 1) * P],
                                 sb_curT[:, j, QS:], start=(j == 0),
                                 stop=(j == 1))
            if g == 0:
                nc.scalar.copy(sb_QT[g][:, QS:], ps[:, 0:NQ - QS])
            else:
                nc.vector.tensor_copy(sb_QT[g][:, QS:], ps[:, 0:NQ - QS])

    # K chunks 1..3 on vps
    w0b = min(3, NCH - 1) * P
    for g in range(2):
        ps = vps[:, 0:w0b]
        for j in range(2):
            nc.tensor.matmul(ps, sb_wk[:, j, g * P:(g + 1) * P],
                             sb_hist[:, j, P:P + w0b],
                             start=(j == 0), stop=(j == 1),
                             skip_group_check=True)
        kt_copy(g, ps, 1, w0b)

    def emit_K(j4):
        # uses the rb bank; all emissions land before the first epilogue
        # broadcast writes rb (K(4) in prologue, rest at t%4==2)
        w = min(4, NCH - j4) * P
        for g in range(2):
            ps = rb[:, 0:w]
            for j in range(2):
                nc.tensor.matmul(ps, sb_wk[:, j, g * P:(g + 1) * P],
                                 sb_hist[:, j, j4 * P:j4 * P + w],
                                 start=(j == 0), stop=(j == 1),
                                 skip_group_check=True)
            kt_copy(g, ps, j4, w)

    # remaining K projections are emitted in early loop steps (t=0,1,2)
    # where the PE pstate is ramped; rb region deps serialize them, and
    # all land before the first epilogue broadcast touches rb (t>=3)

    def emit_V(c):
        ps = vps[:, 0:F]
        for g in range(2):
            nc.tensor.matmul(ps, sb_hist[:, g, c * P:(c + 1) * P],
                             sb_wv[:, g, :], start=(g == 0), stop=(g == 1),
                             skip_group_check=True)
        vv = sb_V[:, c, :].rearrange('p (h x) -> p h x', h=H)
        if c % 2 == 0:
            nc.scalar.copy(vv[:, :, 0:D],
                           ps.rearrange('p (h x) -> p h x', h=H))
        else:
            nc.vector.tensor_copy(vv[:, :, 0:D],
                                  ps.rearrange('p (h x) -> p h x', h=H))

    # ---------------- main loop: software-pipelined over all (slot, chunk)
    av_tiles = {}
    chunks = [(s, j) for s in range(len(KW)) for j in range(KW[s])]
    n = len(chunks)
    sc_tiles = {}
    e_tiles = {}
    pending_epi = []

    def emit_S(t):
        s, j = chunks[t]
        qsl = slice(s * QS, (s + 1) * QS)
        kc = (base[s] + j) * P
        ksl = slice(kc, kc + P)
        # scores: 8 heads, K=32 unmasked, head (g, a) contracts KT/QT rows
        # 32a..32a+32 of group g; banks alternate via a-order (0,2,1,3).
        sc = ps_sc.tile([P, 1024], f32, tag='sc', name='sc')
        scv = sc.rearrange('p (b g c q) -> p b g c q', b=2, g=2, c=2)
        for g in range(2):
            for a in (0, 2, 1, 3):
                b, c = a // 2, a % 2
                nc.tensor.matmul(
                    scv[:, b, g, c, :],
                    sb_KT[g][32 * a:32 * a + 32, ksl],
                    sb_QT[g][32 * a:32 * a + 32, qsl],
                    start=True, stop=True,
                    tile_position=(32 * a, 0))
        sc_tiles[t] = sc

    def emit_EM(t):
        s, j = chunks[t]
        sc = sc_tiles.pop(t)
        e = epool.tile([P, 2, 2, 2, P], b16, tag='e', name='e')
        nc.scalar.activation(e, sc, Exp, scale=ISCALE)
        ef = e.rearrange('p b g c q -> p (b g c) q')
        kc = (base[s] + j) * P
        msl = sb_m01[:, None, kc:kc + P]
        if MSPLIT > 0:
            nc.vector.tensor_tensor(ef[:, 0:MSPLIT, :], ef[:, 0:MSPLIT, :],
                                    msl.to_broadcast([P, MSPLIT, P]),
                                    mybir.AluOpType.mult)
        nc.gpsimd.tensor_tensor(ef[:, MSPLIT:8, :], ef[:, MSPLIT:8, :],
                                msl.to_broadcast([P, 8 - MSPLIT, P]),
                                mybir.AluOpType.mult)
        e_tiles[t] = e

    def emit_AV(t):
        s, j = chunks[t]
        if j == 0:
            # zero the whole av bank and set every has_written bit so the 8
            # interleaved per-head accumulation chains can run start=False;
            # rows 33:64 get 1.0 so the single-op reciprocal over av[32:97]
            # stays finite (those rows' reciprocals are never read).
            av = av_tiles[s] = ps_av.tile([P, 512], f32, tag='av', name='av')
            nc.tensor.matmul(av, sb_init[0:1, :], sb_one512,
                             start=True, stop=False, skip_group_check=True)
        av = av_tiles[s]
        e = e_tiles.pop(t)
        nkc = KW[s]
        for h in range(H):
            g, a = divmod(h, 4)
            po = 64 * (h % 2)
            fo = 128 * (h // 2)
            nc.tensor.matmul(
                av[po:po + 33, fo:fo + QS],
                sb_V[:, base[s] + j, 33 * h:33 * h + 33],
                e[:, a // 2, g, a % 2, :],
                start=False,
                stop=(j == nkc - 1 and h == H - 1),
                skip_group_check=True,
                tile_position=(0, po))
        if j == nkc - 1:
            pending_epi.append([s, 0, 0])

    avsb_tiles = {}
    rbs_tiles = {}
    SLAST = len(KW) - 1

    def emit_epi_a(s):
        # av -> SBUF (ACT), reciprocal of denominator rows (DVE, writing
        # partitions 32/96). Hidden slots broadcast the reciprocal rows via
        # an SBUF->SBUF DMA (latency hides under later chunks); the last
        # slot uses the PE broadcast (short latency for the exposed tail).
        av = av_tiles[s]
        rec = opool.tile([P, 512], b16, tag='rec', name='rec')
        av_sb = opool.tile([P, 512], b16, tag='avsb', name='av_sb')
        with nc.allow_low_precision(reason='softmax denom reciprocal in '
                                    'bf16; rel tol 2e-2 dominates'):
            # one op covers both parities' denominator rows (32 and 96);
            # all other rows are finite (ones-filled / head values) and
            # their reciprocals are never read.
            nc.vector.reciprocal(rec, av)
        nc.scalar.copy(av_sb, av)
        for rho in range(2):
            pr = 32 + 64 * rho
            nc.tensor.matmul(
                rb[64 * rho:64 * rho + D, 0:512],
                sb_ones[pr:pr + 1, 0:D],
                rec[pr:pr + 1, 0:512],
                start=True, stop=True,
                tile_position=(pr, 64 * rho))
        avsb_tiles[s] = av_sb
        av_tiles.pop(s)

    def emit_epi_b(s):
        # normalize: sb_OP = av_sb * rb (single PSUM operand), then the
        # output projection into the spare half of vps
        av_sb = avsb_tiles.pop(s)
        halves = 1
        hq = QS // halves
        for qh in range(halves):
            qsl = slice(s * QS + qh * hq, s * QS + (qh + 1) * hq)
            hs = slice(qh * hq, (qh + 1) * hq)
            for rho in range(2):
                r0 = 64 * rho
                nc.vector.tensor_tensor(
                    sb_OP[r0:r0 + D, :, qsl],
                    av_sb[r0:r0 + D, :].rearrange(
                        'p (b q) -> p b q', b=4)[:, :, hs],
                    rb[r0:r0 + D, :].rearrange(
                        'p (b q) -> p b q', b=4)[:, :, hs],
                    mybir.AluOpType.mult)
            for g2 in range(2):
                ps = vps[:, 256 + 128 * g2:384 + 128 * g2][:, hs]
                for b in range(4):
                    nc.tensor.matmul(ps, sb_woP[:, b, g2 * P:(g2 + 1) * P],
                                     sb_OP[:, b, qsl],
                                     start=(b == 0), stop=(b == 3),
                                     skip_group_check=True)
            if halves == 2:
                emit_epi_c(s, qh, halves)

    def emit_epi_c(s, qh=0, halves=1):
        hq = QS // halves
        for g2 in range(2):
            hs = slice(qh * hq, (qh + 1) * hq)
            qsl = slice(s * QS + qh * hq, s * QS + (qh + 1) * hq)
            ps = vps[:, 256 + 128 * g2:384 + 128 * g2][:, hs]
            y = opool.tile([P, hq], f32, tag=f'y{g2}{qh}', name=f'y{g2}{qh}')
            nc.vector.tensor_copy(y, ps)
            (nc.sync if g2 == 0 else nc.gpsimd).dma_start(
                out=t_yT.ap()[g2 * P:(g2 + 1) * P, qsl], in_=y)

    emit_V(0)
    emit_V(1)
    for t in range(n + 4):
        for ent in pending_epi:
            ent[1] += 1
            if ent[2] == 0 and (ent[1] >= 2 or t >= n):
                emit_epi_a(ent[0])
                ent[2] = 1
        if t < n:
            emit_S(t)
        if 1 <= t <= n:
            emit_EM(t - 1)
        if t == 1:
            emit_Qrest()
            emit_K(4)
        if t == 2:
            for j4 in range(8, NCH, 4):
                emit_K(j4)
        if t + 2 < n:
            emit_V(t + 2)
        if t >= 2 and t - 2 < n:
            emit_AV(t - 2)
        for ent in list(pending_epi):
            if ent[2] == 1 and (ent[1] >= 3 or t >= n + 1):
                emit_epi_b(ent[0])
                emit_epi_c(ent[0])
                pending_epi.remove(ent)


_CACHE = {}


def kernel(**inputs):
    from concourse import bass_utils

    in_maps, qmaps, KW, NKP, vbias = _stage(inputs)
    key = (tuple(KW), vbias)
    if key not in _CACHE:
        _CACHE[key] = _build(KW, NKP, vbias)
    nc = _CACHE[key]
    try:
        res = bass_utils.run_bass_kernel_spmd(nc, in_maps,
                                              core_ids=list(range(NC)))
        yts = [res.results[c]['yT'] for c in range(NC)]
    except Exception:
        # PJRT path unavailable: execute per core on the instruction-level
        # simulator (same program, exact numerics)
        from concourse.bass_interp import CoreSim
        yts = []
        for c in range(NC):
            sim = CoreSim(nc, trace=False, core_id=c, publish_trace=False)
            for name, val in in_maps[c].items():
                sim.tensor(name)[:] = val
            sim.simulate(check_with_hw=False)
            yts.append(np.asarray(sim.tensor('yT')).copy())
    N = inputs['current_feats'].shape[0]
    out = np.zeros((N, F), np.float32)
    for c in range(NC):
        out[qmaps[c]] = yts[c].T
    return out


if __name__ == '__main__':
    pass
